# revision 2
# baseline (speedup 1.0000x reference)
"""Trainium2 Bass kernel for nn_Clustered_Attention_Chunking.

Math notes
----------
The reference computes, with cid = concat(cluster_id, cluster_id):

    out = unsort( self_attention( sort(seq) , mask ) )

where self_attention is applied independently per sequence (each [C=64, E=512]
chunk attends only within itself) and mask is additive.  When the mask is all
zeros (which the fixed `setup_inputs` guarantees: `jnp.zeros`), sorting then
unsorting a batch-independent map is exactly the identity, so the kernel is a
plain batched per-chunk self-attention:

    q = x @ Wq.T ; k = x @ Wk.T ; v = x @ Wv.T        (+ zero biases)
    probs = softmax(q k^T / sqrt(64))  per (seq, head)
    ctx = probs @ v ;  h = ctx @ Wd.T
    out = layernorm(h + x)  with eps inside sqrt, ln_w/ln_b affine

If the mask is ever nonzero we reproduce the reference exactly by doing the
(stable) cluster argsort on the host, feeding sorted sequences to the device
with the mask indexed in *unsorted* order (as the reference does), and
unsorting the result on the host.

Sharding: pure data parallel — 2048 sequences / 8 cores = 256 sequences
(16384 tokens) per core.  No collectives.

v2 changes (vs the DMA-transpose baseline):
  * x^T is pre-transposed (and pre-cast to bf16) on the HOST and fed
    straight from HBM — no SB->SB DMA transposes, no SWDGE cast copies.
    This removes ~650us of DMA-queue churn and the $S stalls that kept
    resetting the PE p-state ramp (PE only reaches 2.4 GHz after ~3us of
    continuous execution).
  * The residual copy of x is bf16 too (halves that DMA stream).
  * Weights arrive pre-transposed AND pre-cast bf16 from the host.
  * DMA prefetch 2 macros deep; projection PSUM pool 3 banks deep so the
    first matmul of each accumulation group never waits on an evacuation.
  * Softmax normalize (probs/rowsum) moved to the idle GPSIMD engine via
    the fused `normalize_recip` ISA op (attn library), freeing ACT/DVE.
  * LayerNorm finalization batched over 4-macro groups so the ACT engine
    swaps its Exp<->Sqrt tables 8x less often (table load = 1.28us each).
"""

import numpy as np

H = 8
E = 512
C = 64
N_FULL = 2048
N_CORES = 8
NSH = N_FULL // N_CORES       # 256 sequences per core
T = NSH * C                   # 16384 tokens per core
TM = 512                      # tokens per macro-block
N_MACRO = T // TM             # 32
GRP = 4                       # macros per layernorm-finalize group
EPS = 1e-12

_CACHE = {}


def _build_program(use_mask, use_bq, use_bk, use_bv, use_bd):
    from contextlib import ExitStack

    import ml_dtypes
    import concourse.bass as bass
    import concourse.mybir as mybir
    import concourse.tile as tile
    from concourse import bacc
    from concourse import library_config

    f32 = mybir.dt.float32
    bf16 = mybir.dt.bfloat16
    AF = mybir.ActivationFunctionType
    ALU = mybir.AluOpType

    nc = bacc.Bacc("TRN2")

    xt_d = nc.dram_tensor("xt", [E, T], bf16, kind="ExternalInput")
    xn_d = nc.dram_tensor("xn", [T, E], bf16, kind="ExternalInput")
    wq_d = nc.dram_tensor("wqt", [E, E], bf16, kind="ExternalInput")
    wk_d = nc.dram_tensor("wkt", [E, E], bf16, kind="ExternalInput")
    wv_d = nc.dram_tensor("wvt", [E, E], bf16, kind="ExternalInput")
    wd_d = nc.dram_tensor("wdt", [E, E], bf16, kind="ExternalInput")
    out_d = nc.dram_tensor("out", [T, E], f32, kind="ExternalOutput")
    mask_d = bq_d = bk_d = bv_d = bd_d = None
    if use_mask:
        mask_d = nc.dram_tensor("mask", [T, C], f32, kind="ExternalInput")
    if use_bq:
        bq_d = nc.dram_tensor("bq", [E], f32, kind="ExternalInput")
    if use_bk:
        bk_d = nc.dram_tensor("bk", [E], f32, kind="ExternalInput")
    if use_bv:
        bv_d = nc.dram_tensor("bv", [E], f32, kind="ExternalInput")
    if use_bd:
        bd_d = nc.dram_tensor("bdb", [128, E], f32, kind="ExternalInput")

    id64_np = np.tile(np.eye(64, dtype=np.float32), (2, 1)).astype(ml_dtypes.bfloat16)
    id64_d = nc.inline_tensor(id64_np, name="id64")

    with tile.TileContext(nc) as tc, ExitStack() as ctx:
        nc.gpsimd.load_library(library_config.attn)

        consts = ctx.enter_context(tc.tile_pool(name="consts", bufs=1))

        # Weights, pre-transposed + bf16 on host: w*T[e, e'] = W[e', e];
        # tiled [p, a, e'] with row index e = a*128 + p.
        w_sb = {}
        for nm, dd in (("q", wq_d), ("k", wk_d), ("v", wv_d), ("d", wd_d)):
            t = consts.tile([128, 4, E], bf16, tag=f"w{nm}", name=f"w{nm}")
            nc.sync.dma_start(t[:], dd[:].rearrange("(a p) e -> p a e", p=128))
            w_sb[nm] = t

        id64 = consts.tile([128, 64], bf16, tag="id64", name="id64")
        nc.sync.dma_start(id64[:], id64_d[:])
        eps_t = consts.tile([128, 1], f32, tag="eps", name="eps")
        nc.vector.memset(eps_t[:], EPS)

        bias_sb = {}
        for nm, dd in (("q", bq_d), ("k", bk_d), ("v", bv_d)):
            if dd is not None:
                t = consts.tile([128, 4], f32, tag=f"b{nm}", name=f"b{nm}")
                nc.sync.dma_start(t[:], dd[:].rearrange("(a p) -> p a", p=128))
                bias_sb[nm] = t
        if bd_d is not None:
            t = consts.tile([128, E], f32, tag="bd", name="bd")
            nc.sync.dma_start(t[:], bd_d[:])
            bias_sb["d"] = t

        # SBUF pools
        p_xt = ctx.enter_context(tc.tile_pool(name="p_xt", bufs=3))
        p_xn = ctx.enter_context(tc.tile_pool(name="p_xn", bufs=3))
        p_qk = ctx.enter_context(tc.tile_pool(name="p_qk", bufs=16))
        p_v = ctx.enter_context(tc.tile_pool(name="p_v", bufs=8))
        p_ct = ctx.enter_context(tc.tile_pool(name="p_ct", bufs=2))
        p_pb = ctx.enter_context(tc.tile_pool(name="p_pb", bufs=4))
        p_pn = ctx.enter_context(tc.tile_pool(name="p_pn", bufs=4))
        p_pt = ctx.enter_context(tc.tile_pool(name="p_pt", bufs=8))
        p_sm = ctx.enter_context(tc.tile_pool(name="p_sm", bufs=8))
        p_ln = ctx.enter_context(tc.tile_pool(name="p_ln", bufs=8))
        p_h = ctx.enter_context(tc.tile_pool(name="p_h", bufs=20))
        p_o = ctx.enter_context(tc.tile_pool(name="p_o", bufs=8))
        p_msk = (
            ctx.enter_context(tc.tile_pool(name="p_msk", bufs=12)) if use_mask else None
        )

        # PSUM pools: pp = [128,512] f32 (1 bank) x3; pa = [128,256] x5
        pp = ctx.enter_context(tc.tile_pool(name="pp", bufs=3, space="PSUM"))
        pa = ctx.enter_context(tc.tile_pool(name="pa", bufs=5, space="PSUM"))

        def prep(m):
            """Issue all HBM loads for macro m: xT (bf16, host-pretransposed)
            and x natural (bf16, residual only), plus the mask."""
            t0 = m * TM
            xt = p_xt.tile([128, 4, TM], bf16, tag="xt", name="xt")
            for ec in range(4):
                nc.sync.dma_start(
                    xt[:, ec, :], xt_d[ec * 128 : (ec + 1) * 128, t0 : t0 + TM]
                )
            xn = p_xn.tile([128, 4, E], bf16, tag="xn", name="xn")
            for t4 in range(4):
                nc.sync.dma_start(
                    xn[:, t4, :], xn_d[t0 + t4 * 128 : t0 + (t4 + 1) * 128, :]
                )
            msk = []
            if use_mask:
                for t4 in range(4):
                    mt = p_msk.tile([128, C], f32, tag="msk", name="msk")
                    nc.sync.dma_start(
                        mt[:], mask_d[t0 + t4 * 128 : t0 + (t4 + 1) * 128, :]
                    )
                    msk.append(mt)
            return xt, xn, msk

        def do_proj(m, prep_res):
            """qT/kT (transposed, weights stationary) and v (natural, xT
            stationary) projections for macro m."""
            xt, xn, msk = prep_res
            qT, kT = [], []
            for nm, dst in (("q", qT), ("k", kT)):
                for c in range(4):
                    ps = pp.tile([128, TM], f32, tag="proj", name="proj")
                    for ec in range(4):
                        nc.tensor.matmul(
                            ps[:],
                            w_sb[nm][:, ec, c * 128 : (c + 1) * 128],
                            xt[:, ec, :],
                            start=(ec == 0),
                            stop=(ec == 3),
                        )
                    sb = p_qk.tile([128, TM], bf16, tag=f"{nm}T", name=f"{nm}T")
                    if nm in bias_sb:
                        nc.scalar.activation(
                            sb[:], ps[:], AF.Identity, bias=bias_sb[nm][:, c : c + 1]
                        )
                    elif c % 2 == 0:
                        nc.scalar.copy(sb[:], ps[:])
                    else:
                        nc.vector.tensor_copy(sb[:], ps[:])
                    dst.append(sb)
            v_nat = []
            for t4 in range(4):
                ps = pp.tile([128, TM], f32, tag="proj", name="proj")
                for ec in range(4):
                    nc.tensor.matmul(
                        ps[:],
                        xt[:, ec, t4 * 128 : (t4 + 1) * 128],
                        w_sb["v"][:, ec, :],
                        start=(ec == 0),
                        stop=(ec == 3),
                    )
                sb = p_v.tile([128, TM], bf16, tag="v", name="v")
                if t4 % 2 == 0:
                    nc.scalar.copy(sb[:], ps[:])
                else:
                    nc.vector.tensor_copy(sb[:], ps[:])
                v_nat.append(sb)
            # (bv is folded in after the ctx matmul: sum_j probs = 1.)
            return xn, msk, qT, kT, v_nat

        # Group-batched layernorm state
        hs_all = {}       # (m, t4) -> h tile
        mv_grp = {}       # g -> [128, 2, 16] mean/var tile

        def finalize_group(g):
            """Batched LN tail for macros 4g..4g+3: one ACT sqrt per group
            (instead of per macro) so the Exp<->Sqrt table churn drops 4x."""
            mv = mv_grp.pop(g)
            std = p_ln.tile([128, 16], f32, tag="std", name="std")
            nc.scalar.activation(
                std[:], mv[:, 1, :], AF.Sqrt, bias=eps_t[:, 0:1], scale=1.0
            )
            rstd = p_ln.tile([128, 16], f32, tag="rstd", name="rstd")
            nc.vector.reciprocal(rstd[:], std[:])
            negmr = p_ln.tile([128, 16], f32, tag="negmr", name="negmr")
            nc.vector.tensor_mul(negmr[:], mv[:, 0, :], rstd[:])
            for gi in range(GRP):
                m = g * GRP + gi
                t0 = m * TM
                for t4 in range(4):
                    idx = gi * 4 + t4
                    h = hs_all.pop((m, t4))
                    o = p_o.tile([128, E], f32, tag="o", name="o")
                    nc.vector.tensor_scalar(
                        o[:],
                        h[:],
                        rstd[:, idx : idx + 1],
                        negmr[:, idx : idx + 1],
                        ALU.mult,
                        ALU.subtract,
                    )
                    nc.sync.dma_start(
                        out_d[t0 + t4 * 128 : t0 + (t4 + 1) * 128, :], o[:]
                    )

        prepped = [prep(0), prep(1)]
        nxt = do_proj(0, prepped.pop(0))
        for m in range(N_MACRO):
            t0 = m * TM
            xn, msk, qT, kT, v_nat = nxt
            # emit next macro's DMAs + projections now: their copies drain on
            # ACT/DVE while this macro's attention runs, and the PE goes from
            # this macro's attention straight into next macro's projections.
            if m + 2 < N_MACRO:
                prepped.append(prep(m + 2))
            if m + 1 < N_MACRO:
                nxt = do_proj(m + 1, prepped.pop(0))
            # previous group's LN tail lands here: its DVE burst overlaps the
            # PE running next macro's projections.
            if m % GRP == 0 and m >= GRP:
                finalize_group(m // GRP - 1)

            # ---- attention: 4 block-pairs (128 tokens); all 4 head-pair
            # chunks of a block-pair share fused PSUM tiles ("quad" scheme).
            # Safety rules: concurrent packed MMs sharing a col-group but
            # differing in row-group must hit different banks; MMs at the
            # same (row, col) position serialize in hardware and may share.
            ctxT = p_ct.tile([128, 4, TM], bf16, tag="ctxT", name="ctxT")

            def scores_softmax(p4):
                # scores natural: ps_s[hb] layout [i(sb-packed), (c, j)]
                ps_s = [
                    pa.tile([128, 4, 64], f32, tag="small", name="ps_s")
                    for _ in (0, 1)
                ]
                for hb in (0, 1):
                    for c in range(4):
                        for sb_ in (0, 1):
                            tsl = slice(
                                p4 * 128 + sb_ * 64, p4 * 128 + (sb_ + 1) * 64
                            )
                            hsl = slice(hb * 64, (hb + 1) * 64)
                            nc.tensor.matmul(
                                ps_s[hb][sb_ * 64 : (sb_ + 1) * 64, c, :],
                                qT[c][hsl, tsl],
                                kT[c][hsl, tsl],
                                start=True,
                                stop=True,
                            )
                if use_mask:
                    for hb in (0, 1):
                        for c in range(4):
                            nc.vector.tensor_add(
                                ps_s[hb][:, c, :], ps_s[hb][:, c, :], msk[p4][:]
                            )
                # exp (scale=1/8) + row sums; probs: [128, hb, c, j] f32
                probs = p_pb.tile([128, 2, 4, 64], f32, tag="probs", name="probs")
                for hb in (0, 1):
                    nc.scalar.activation(
                        probs[:, hb, :, :], ps_s[hb][:], AF.Exp, scale=0.125
                    )
                sums = p_sm.tile([128, 2, 4], f32, tag="sums", name="sums")
                nc.vector.tensor_reduce(
                    sums[:],
                    probs[:],
                    axis=mybir.AxisListType.X,
                    op=ALU.add,
                )
                # fused normalize + reciprocal on the (otherwise idle) GPSIMD
                pn = p_pn.tile([128, 2, 4, 64], bf16, tag="pn", name="pn")
                for hb in (0, 1):
                    for c in range(4):
                        nc.gpsimd.normalize_recip(
                            pn[:, hb, c, :],
                            probs[:, hb, c, :],
                            sums[:, hb, c : c + 1],
                        )
                return pn

            def trans(p4, pn):
                # transpose 64x64 blocks (PE): pT[hb] layout [j(sb), (c, i)]
                ps_pt = [
                    pa.tile([128, 4, 64], bf16, tag="small", name="ps_pt")
                    for _ in (0, 1)
                ]
                for hb in (0, 1):
                    for c in range(4):
                        for sb_ in (0, 1):
                            ssl = slice(sb_ * 64, (sb_ + 1) * 64)
                            nc.tensor.transpose(
                                ps_pt[hb][ssl, c, :],
                                pn[ssl, hb, c, :],
                                id64[ssl, :],
                            )
                pts = [
                    p_pt.tile([128, 4, 64], bf16, tag="pts", name="pts")
                    for _ in (0, 1)
                ]
                for hb in (0, 1):
                    nc.vector.tensor_copy(pts[hb][:], ps_pt[hb][:])
                return pts

            def ctx_out(p4, pts):
                # ctx^T: ps_c[sb] layout [d(hb-packed), (c, i of sb)]
                ps_c = [
                    pa.tile([128, 4, 64], f32, tag="small", name="ps_c")
                    for _ in (0, 1)
                ]
                for sb_ in (0, 1):
                    ssl = slice(sb_ * 64, (sb_ + 1) * 64)
                    for c in range(4):
                        for hb in (0, 1):
                            hsl = slice(hb * 64, (hb + 1) * 64)
                            nc.tensor.matmul(
                                ps_c[sb_][hsl, c, :],
                                v_nat[p4][ssl, (2 * c + hb) * 64 : (2 * c + hb + 1) * 64],
                                pts[hb][ssl, c, :],
                                start=True,
                                stop=True,
                            )
                for sb_ in (0, 1):
                    dst = ctxT[:, :, p4 * 128 + sb_ * 64 : p4 * 128 + (sb_ + 1) * 64]
                    if "v" in bias_sb:
                        # bv differs per chunk c (partition meaning changes
                        # with c) -> per-chunk copies on the bias path
                        for c in range(4):
                            nc.scalar.activation(
                                dst[:, c, :],
                                ps_c[sb_][:, c, :],
                                AF.Identity,
                                bias=bias_sb["v"][:, c : c + 1],
                            )
                    else:
                        nc.scalar.copy(dst, ps_c[sb_][:])

            # software-pipelined, 2-deep skew: PE runs scores(p4),
            # transposes(p4-1), ctx(p4-2) back to back so it never waits on
            # the ACT/DVE softmax chain.
            pn_l = [None] * 4
            pts_l = [None] * 4
            for p4 in range(4):
                pn_l[p4] = scores_softmax(p4)
                if p4 >= 1:
                    pts_l[p4 - 1] = trans(p4 - 1, pn_l[p4 - 1])
                if p4 >= 2:
                    ctx_out(p4 - 2, pts_l[p4 - 2])
            pts_l[3] = trans(3, pn_l[3])
            ctx_out(2, pts_l[2])
            ctx_out(3, pts_l[3])

            # ---- output projection + residual; layernorm stats only (the
            # sqrt + final scale are batched per 4-macro group).
            g = m // GRP
            if m % GRP == 0:
                mv_grp[g] = p_ln.tile([128, 2, 16], f32, tag="mv", name="mv")
            mv = mv_grp[g]
            for t4 in range(4):
                ps_o = pp.tile([128, E], f32, tag="proj", name="proj")
                for c in range(4):
                    nc.tensor.matmul(
                        ps_o[:],
                        ctxT[:, c, t4 * 128 : (t4 + 1) * 128],
                        w_sb["d"][:, c, :],
                        start=(c == 0),
                        stop=(c == 3),
                    )
                h = p_h.tile([128, E], f32, tag="h", name="h")
                nc.vector.tensor_add(h[:], ps_o[:], xn[:, t4, :])
                if "d" in bias_sb:
                    nc.vector.tensor_add(h[:], h[:], bias_sb["d"][:])
                hs_all[(m, t4)] = h
                stats = p_sm.tile([128, 6], f32, tag="stats", name="stats")
                nc.vector.bn_stats(stats[:], h[:])
                idx = (m % GRP) * 4 + t4
                nc.vector.bn_aggr(mv[:, :, idx : idx + 1], stats[:])

        finalize_group(N_MACRO // GRP - 1)

    nc.compile()
    return nc


def _ensure_ntff_hook():
    """bass_utils' trace path does `from antenv.axon_hooks import ...`,
    which this container's antenv lacks.  Provide it, wired to the axon
    PJRT .so via ctypes (mirrors trn_agent_boot._ntff_profile_via_ctypes),
    so trace=True works; degrade to a None hook otherwise."""
    import sys
    import types

    try:
        import antenv.axon_hooks  # noqa: F401

        return
    except ImportError:
        pass
    mod = types.ModuleType("antenv.axon_hooks")
    state = {"hook": None}
    mod.set_axon_ntff_profile_hook = lambda h: state.__setitem__("hook", h)
    mod.get_axon_ntff_profile_hook = lambda: state["hook"]
    try:
        import antenv

        antenv.axon_hooks = mod
    except ImportError:
        pass
    sys.modules["antenv.axon_hooks"] = mod

    so_path = "/opt/axon/libaxon_pjrt.so"
    try:
        import importlib.util
        import os

        boot_py = None
        for base in (os.environ.get("AXON_SITE_DIR", "/root/.axon_site"),):
            cand = os.path.join(base, "trn_agent_boot", "trn_boot.py")
            if os.path.exists(cand):
                boot_py = cand
        if boot_py and os.path.exists(so_path):
            spec = importlib.util.spec_from_file_location("_trn_boot_hook", boot_py)
            tb = importlib.util.module_from_spec(spec)
            spec.loader.exec_module(tb)
            state["hook"] = tb._ntff_profile_via_ctypes(so_path)
    except Exception:
        state["hook"] = None


def kernel(
    seq,
    attention_mask,
    cluster_id,
    Wq,
    bq,
    Wk,
    bk,
    Wv,
    bv,
    Wd,
    bd,
    ln_w,
    ln_b,
):
    _ensure_ntff_hook()
    import ml_dtypes
    import concourse.bass_utils as bass_utils

    seq = np.ascontiguousarray(np.asarray(seq, dtype=np.float32))
    attention_mask = np.asarray(attention_mask, dtype=np.float32)
    use_mask = bool(np.any(attention_mask))
    Wq = np.asarray(Wq, np.float32)
    Wk = np.asarray(Wk, np.float32)
    Wv = np.asarray(Wv, np.float32)
    Wd = np.asarray(Wd, np.float32)
    bq = np.asarray(bq, np.float32)
    bk = np.asarray(bk, np.float32)
    bv = np.asarray(bv, np.float32)
    bd = np.asarray(bd, np.float32)
    ln_w = np.asarray(ln_w, np.float32)
    ln_b = np.asarray(ln_b, np.float32)
    use_bq, use_bk = bool(np.any(bq)), bool(np.any(bk))
    use_bv, use_bd = bool(np.any(bv)), bool(np.any(bd))

    key = (use_mask, use_bq, use_bk, use_bv, use_bd)
    if key not in _CACHE:
        _CACHE[key] = _build_program(*key)
    nc = _CACHE[key]

    if use_mask:
        # Reproduce the reference exactly: sort sequences by cluster id
        # (stable, as jnp.argsort), keep mask in unsorted order.
        cid2 = np.concatenate([np.asarray(cluster_id), np.asarray(cluster_id)])
        sidx = np.argsort(cid2, kind="stable")
        xs = seq[sidx]
    else:
        xs = seq  # sort o unsort == identity for batch-independent attention

    x_flat = xs.reshape(N_FULL * C, E)
    bf = ml_dtypes.bfloat16
    base = {
        "wqt": np.ascontiguousarray(Wq.T).astype(bf),
        "wkt": np.ascontiguousarray(Wk.T).astype(bf),
        "wvt": np.ascontiguousarray(Wv.T).astype(bf),
        "wdt": np.ascontiguousarray(Wd.T).astype(bf),
    }
    if use_bq:
        base["bq"] = bq
    if use_bk:
        base["bk"] = bk
    if use_bv:
        base["bv"] = bv
    if use_bd:
        base["bdb"] = np.ascontiguousarray(np.tile(bd[None, :], (128, 1)))
    in_maps = []
    for i in range(N_CORES):
        im = dict(base)
        xc = x_flat[i * T : (i + 1) * T]
        xb = xc.astype(bf)
        im["xn"] = np.ascontiguousarray(xb)
        im["xt"] = np.ascontiguousarray(xb.T)
        if use_mask:
            im["mask"] = np.ascontiguousarray(
                attention_mask[i * NSH : (i + 1) * NSH, 0, :, :].reshape(T, C)
            )
        in_maps.append(im)

    import os

    trace = bool(int(os.environ.get("KERNEL_TRACE", "0")))
    res = bass_utils.run_bass_kernel_spmd(
        nc, in_maps, core_ids=list(range(N_CORES)), trace=trace
    )
    kernel._last_result = res

    out = np.concatenate([r["out"] for r in res.results], axis=0)
    out = out.reshape(N_FULL, C, E)
    if use_mask:
        out = out[np.argsort(sidx, kind="stable")]
    if not (np.all(ln_w == 1.0) and np.all(ln_b == 0.0)):
        out = out * ln_w + ln_b
    return out.astype(np.float32)


# revision 6
# speedup vs baseline: 1.0006x; 1.0006x over previous
"""Trainium2 Bass kernel for nn_Clustered_Attention_Chunking.

Math notes
----------
The reference computes, with cid = concat(cluster_id, cluster_id):

    out = unsort( self_attention( sort(seq) , mask ) )

where self_attention is applied independently per sequence (each [C=64, E=512]
chunk attends only within itself) and mask is additive.  When the mask is all
zeros (which the fixed `setup_inputs` guarantees: `jnp.zeros`), sorting then
unsorting a batch-independent map is exactly the identity, so the kernel is a
plain batched per-chunk self-attention:

    q = x @ Wq.T ; k = x @ Wk.T ; v = x @ Wv.T        (+ zero biases)
    probs = softmax(q k^T / sqrt(64))  per (seq, head)
    ctx = probs @ v ;  h = ctx @ Wd.T
    out = layernorm(h + x)  with eps inside sqrt, ln_w/ln_b affine

If the mask is ever nonzero we reproduce the reference exactly by doing the
(stable) cluster argsort on the host, feeding sorted sequences to the device
with the mask indexed in *unsorted* order (as the reference does), and
unsorting the result on the host.

Sharding: pure data parallel — 2048 sequences / 8 cores = 256 sequences
(16384 tokens) per core.  No collectives.

v2 changes (vs the DMA-transpose baseline):
  * x^T is pre-transposed (and pre-cast to bf16) on the HOST and fed
    straight from HBM — no SB->SB DMA transposes, no SWDGE cast copies.
    This removes ~650us of DMA-queue churn and the $S stalls that kept
    resetting the PE p-state ramp (PE only reaches 2.4 GHz after ~3us of
    continuous execution).
  * The residual copy of x is bf16 too (halves that DMA stream).
  * Weights arrive pre-transposed AND pre-cast bf16 from the host.
  * DMA prefetch 2 macros deep; projection PSUM pool 3 banks deep so the
    first matmul of each accumulation group never waits on an evacuation.
  * Softmax normalize (probs/rowsum) moved to the idle GPSIMD engine via
    the fused `normalize_recip` ISA op (attn library), freeing ACT/DVE.
  * LayerNorm finalization batched over 4-macro groups so the ACT engine
    swaps its Exp<->Sqrt tables 8x less often (table load = 1.28us each).
"""

import numpy as np

H = 8
E = 512
C = 64
N_FULL = 2048
N_CORES = 8
NSH = N_FULL // N_CORES       # 256 sequences per core
T = NSH * C                   # 16384 tokens per core
TM = 512                      # tokens per macro-block
N_MACRO = T // TM             # 32
GRP = 4                       # macros per layernorm-finalize group
EPS = 1e-12

_CACHE = {}


def _build_program(use_mask, use_bq, use_bk, use_bv, use_bd):
    from contextlib import ExitStack

    import ml_dtypes
    import concourse.bass as bass
    import concourse.mybir as mybir
    import concourse.tile as tile
    from concourse import bacc
    from concourse import library_config

    f32 = mybir.dt.float32
    bf16 = mybir.dt.bfloat16
    AF = mybir.ActivationFunctionType
    ALU = mybir.AluOpType

    nc = bacc.Bacc("TRN2")

    xt_d = nc.dram_tensor("xt", [E, T], bf16, kind="ExternalInput")
    xn_d = nc.dram_tensor("xn", [T, E], bf16, kind="ExternalInput")
    wq_d = nc.dram_tensor("wqt", [E, E], bf16, kind="ExternalInput")
    wk_d = nc.dram_tensor("wkt", [E, E], bf16, kind="ExternalInput")
    wv_d = nc.dram_tensor("wvt", [E, E], bf16, kind="ExternalInput")
    wd_d = nc.dram_tensor("wdt", [E, E], bf16, kind="ExternalInput")
    out_d = nc.dram_tensor("out", [T, E], f32, kind="ExternalOutput")
    mask_d = bq_d = bk_d = bv_d = bd_d = None
    if use_mask:
        mask_d = nc.dram_tensor("mask", [T, C], f32, kind="ExternalInput")
    if use_bq:
        bq_d = nc.dram_tensor("bq", [E], f32, kind="ExternalInput")
    if use_bk:
        bk_d = nc.dram_tensor("bk", [E], f32, kind="ExternalInput")
    if use_bv:
        bv_d = nc.dram_tensor("bv", [E], f32, kind="ExternalInput")
    if use_bd:
        bd_d = nc.dram_tensor("bdb", [128, E], f32, kind="ExternalInput")

    id64_np = np.tile(np.eye(64, dtype=np.float32), (2, 1)).astype(ml_dtypes.bfloat16)
    id64_d = nc.inline_tensor(id64_np, name="id64")

    with tile.TileContext(nc) as tc, ExitStack() as ctx:
        nc.gpsimd.load_library(library_config.attn)

        consts = ctx.enter_context(tc.tile_pool(name="consts", bufs=1))

        # Weights, pre-transposed + bf16 on host: w*T[e, e'] = W[e', e];
        # tiled [p, a, e'] with row index e = a*128 + p.
        w_sb = {}
        for nm, dd in (("q", wq_d), ("k", wk_d), ("v", wv_d), ("d", wd_d)):
            t = consts.tile([128, 4, E], bf16, tag=f"w{nm}", name=f"w{nm}")
            nc.sync.dma_start(t[:], dd[:].rearrange("(a p) e -> p a e", p=128))
            w_sb[nm] = t

        id64 = consts.tile([128, 64], bf16, tag="id64", name="id64")
        nc.sync.dma_start(id64[:], id64_d[:])
        eps_t = consts.tile([128, 1], f32, tag="eps", name="eps")
        nc.vector.memset(eps_t[:], EPS)

        bias_sb = {}
        for nm, dd in (("q", bq_d), ("k", bk_d), ("v", bv_d)):
            if dd is not None:
                t = consts.tile([128, 4], f32, tag=f"b{nm}", name=f"b{nm}")
                nc.sync.dma_start(t[:], dd[:].rearrange("(a p) -> p a", p=128))
                bias_sb[nm] = t
        if bd_d is not None:
            t = consts.tile([128, E], f32, tag="bd", name="bd")
            nc.sync.dma_start(t[:], bd_d[:])
            bias_sb["d"] = t

        # SBUF pools
        p_xt = ctx.enter_context(tc.tile_pool(name="p_xt", bufs=3))
        p_xn = ctx.enter_context(tc.tile_pool(name="p_xn", bufs=3))
        p_qk = ctx.enter_context(tc.tile_pool(name="p_qk", bufs=16))
        p_v = ctx.enter_context(tc.tile_pool(name="p_v", bufs=8))
        p_ct = ctx.enter_context(tc.tile_pool(name="p_ct", bufs=2))
        p_pb = ctx.enter_context(tc.tile_pool(name="p_pb", bufs=4))
        p_pn = ctx.enter_context(tc.tile_pool(name="p_pn", bufs=4))
        p_pt = ctx.enter_context(tc.tile_pool(name="p_pt", bufs=8))
        p_sm = ctx.enter_context(tc.tile_pool(name="p_sm", bufs=8))
        p_ln = ctx.enter_context(tc.tile_pool(name="p_ln", bufs=8))
        p_h = ctx.enter_context(tc.tile_pool(name="p_h", bufs=20))
        p_o = ctx.enter_context(tc.tile_pool(name="p_o", bufs=8))
        p_msk = (
            ctx.enter_context(tc.tile_pool(name="p_msk", bufs=12)) if use_mask else None
        )

        # PSUM pools: pp = [128,512] f32 (1 bank) x3; pa = [128,256] x5
        pp = ctx.enter_context(tc.tile_pool(name="pp", bufs=3, space="PSUM"))
        pa = ctx.enter_context(tc.tile_pool(name="pa", bufs=5, space="PSUM"))

        def prep(m):
            """Issue all HBM loads for macro m: xT (bf16, host-pretransposed)
            and x natural (bf16, residual only), plus the mask."""
            t0 = m * TM
            xt = p_xt.tile([128, 4, TM], bf16, tag="xt", name="xt")
            for ec in range(4):
                nc.sync.dma_start(
                    xt[:, ec, :], xt_d[ec * 128 : (ec + 1) * 128, t0 : t0 + TM]
                )
            xn = p_xn.tile([128, 4, E], bf16, tag="xn", name="xn")
            for t4 in range(4):
                nc.sync.dma_start(
                    xn[:, t4, :], xn_d[t0 + t4 * 128 : t0 + (t4 + 1) * 128, :]
                )
            msk = []
            if use_mask:
                for t4 in range(4):
                    mt = p_msk.tile([128, C], f32, tag="msk", name="msk")
                    nc.sync.dma_start(
                        mt[:], mask_d[t0 + t4 * 128 : t0 + (t4 + 1) * 128, :]
                    )
                    msk.append(mt)
            return xt, xn, msk

        def do_proj(m, prep_res):
            """qT/kT (transposed, weights stationary) and v (natural, xT
            stationary) projections for macro m."""
            xt, xn, msk = prep_res
            qT, kT = [], []
            for nm, dst in (("q", qT), ("k", kT)):
                for c in range(4):
                    ps = pp.tile([128, TM], f32, tag="proj", name="proj")
                    for ec in range(4):
                        nc.tensor.matmul(
                            ps[:],
                            w_sb[nm][:, ec, c * 128 : (c + 1) * 128],
                            xt[:, ec, :],
                            start=(ec == 0),
                            stop=(ec == 3),
                        )
                    sb = p_qk.tile([128, TM], bf16, tag=f"{nm}T", name=f"{nm}T")
                    if nm in bias_sb:
                        nc.scalar.activation(
                            sb[:], ps[:], AF.Identity, bias=bias_sb[nm][:, c : c + 1]
                        )
                    elif c % 2 == 0:
                        nc.scalar.copy(sb[:], ps[:])
                    else:
                        nc.vector.tensor_copy(sb[:], ps[:])
                    dst.append(sb)
            v_nat = []
            for t4 in range(4):
                ps = pp.tile([128, TM], f32, tag="proj", name="proj")
                for ec in range(4):
                    nc.tensor.matmul(
                        ps[:],
                        xt[:, ec, t4 * 128 : (t4 + 1) * 128],
                        w_sb["v"][:, ec, :],
                        start=(ec == 0),
                        stop=(ec == 3),
                    )
                sb = p_v.tile([128, TM], bf16, tag="v", name="v")
                nc.scalar.copy(sb[:], ps[:])
                v_nat.append(sb)
            # (bv is folded in after the ctx matmul: sum_j probs = 1.)
            return xn, msk, qT, kT, v_nat

        # Group-batched layernorm state
        hs_all = {}       # (m, t4) -> h tile
        mv_grp = {}       # g -> [128, 2, 16] mean/var tile
        ln_grp = {}       # g -> (rstd, negmr) tiles

        def ln_stats_group(g):
            """Batched LN sqrt for macros 4g..4g+3: one ACT sqrt per group
            (instead of per macro) so the Exp<->Sqrt table churn drops 4x."""
            mv = mv_grp.pop(g)
            std = p_ln.tile([128, 16], f32, tag="std", name="std")
            nc.scalar.activation(
                std[:], mv[:, 1, :], AF.Sqrt, bias=eps_t[:, 0:1], scale=1.0
            )
            rstd = p_ln.tile([128, 16], f32, tag="rstd", name="rstd")
            nc.vector.reciprocal(rstd[:], std[:])
            negmr = p_ln.tile([128, 16], f32, tag="negmr", name="negmr")
            nc.vector.tensor_mul(negmr[:], mv[:, 0, :], rstd[:])
            ln_grp[g] = (rstd, negmr)

        def finalize_macro(m):
            """Final LN scale + store for ONE past macro (spread over the
            following iterations so the DVE never gets a >3.4us burst that
            would stall the softmax chain and HAM-rethrottle the PE)."""
            g, gi = m // GRP, m % GRP
            rstd, negmr = ln_grp[g]
            t0 = m * TM
            for t4 in range(4):
                idx = gi * 4 + t4
                h = hs_all.pop((m, t4))
                o = p_o.tile([128, E], f32, tag="o", name="o")
                nc.vector.tensor_scalar(
                    o[:],
                    h[:],
                    rstd[:, idx : idx + 1],
                    negmr[:, idx : idx + 1],
                    ALU.mult,
                    ALU.subtract,
                )
                nc.sync.dma_start(
                    out_d[t0 + t4 * 128 : t0 + (t4 + 1) * 128, :], o[:]
                )

        prepped = [prep(0), prep(1)]
        nxt = do_proj(0, prepped.pop(0))
        for m in range(N_MACRO):
            t0 = m * TM
            xn, msk, qT, kT, v_nat = nxt
            # emit next macro's DMAs + projections now: their copies drain on
            # ACT/DVE while this macro's attention runs, and the PE goes from
            # this macro's attention straight into next macro's projections.
            if m + 2 < N_MACRO:
                prepped.append(prep(m + 2))
            if m + 1 < N_MACRO:
                nxt = do_proj(m + 1, prepped.pop(0))
            # LN tail for macro m-GRP lands here: one macro's worth per
            # iteration so the DVE burst stays small, overlapping the PE
            # running next macro's projections.
            if m % GRP == 0 and m >= GRP:
                ln_stats_group(m // GRP - 1)
            if m >= GRP:
                finalize_macro(m - GRP)

            # ---- attention: 4 block-pairs (128 tokens); all 4 head-pair
            # chunks of a block-pair share fused PSUM tiles ("quad" scheme).
            # Safety rules: concurrent packed MMs sharing a col-group but
            # differing in row-group must hit different banks; MMs at the
            # same (row, col) position serialize in hardware and may share.
            ctxT = p_ct.tile([128, 4, TM], bf16, tag="ctxT", name="ctxT")

            def scores_softmax(p4):
                # scores natural: ps_s[hb] layout [i(sb-packed), (c, j)]
                ps_s = [
                    pa.tile([128, 4, 64], f32, tag="small", name="ps_s")
                    for _ in (0, 1)
                ]
                for hb in (0, 1):
                    for c in range(4):
                        for sb_ in (0, 1):
                            tsl = slice(
                                p4 * 128 + sb_ * 64, p4 * 128 + (sb_ + 1) * 64
                            )
                            hsl = slice(hb * 64, (hb + 1) * 64)
                            nc.tensor.matmul(
                                ps_s[hb][sb_ * 64 : (sb_ + 1) * 64, c, :],
                                qT[c][hsl, tsl],
                                kT[c][hsl, tsl],
                                start=True,
                                stop=True,
                            )
                if use_mask:
                    for hb in (0, 1):
                        for c in range(4):
                            nc.vector.tensor_add(
                                ps_s[hb][:, c, :], ps_s[hb][:, c, :], msk[p4][:]
                            )
                # exp (scale=1/8) + row sums; probs: [128, hb, c, j] f32
                probs = p_pb.tile([128, 2, 4, 64], f32, tag="probs", name="probs")
                for hb in (0, 1):
                    nc.scalar.activation(
                        probs[:, hb, :, :], ps_s[hb][:], AF.Exp, scale=0.125
                    )
                sums = p_sm.tile([128, 2, 4], f32, tag="sums", name="sums")
                nc.vector.tensor_reduce(
                    sums[:],
                    probs[:],
                    axis=mybir.AxisListType.X,
                    op=ALU.add,
                )
                # fused normalize + reciprocal on the (otherwise idle) GPSIMD
                pn = p_pn.tile([128, 2, 4, 64], bf16, tag="pn", name="pn")
                for hb in (0, 1):
                    for c in range(4):
                        nc.gpsimd.normalize_recip(
                            pn[:, hb, c, :],
                            probs[:, hb, c, :],
                            sums[:, hb, c : c + 1],
                        )
                return pn

            def trans(p4, pn):
                # transpose 64x64 blocks (PE): pT[hb] layout [j(sb), (c, i)]
                ps_pt = [
                    pa.tile([128, 4, 64], bf16, tag="small", name="ps_pt")
                    for _ in (0, 1)
                ]
                for hb in (0, 1):
                    for c in range(4):
                        for sb_ in (0, 1):
                            ssl = slice(sb_ * 64, (sb_ + 1) * 64)
                            nc.tensor.transpose(
                                ps_pt[hb][ssl, c, :],
                                pn[ssl, hb, c, :],
                                id64[ssl, :],
                            )
                pts = [
                    p_pt.tile([128, 4, 64], bf16, tag="pts", name="pts")
                    for _ in (0, 1)
                ]
                for hb in (0, 1):
                    nc.vector.tensor_copy(pts[hb][:], ps_pt[hb][:])
                return pts

            def ctx_out(p4, pts):
                # ctx^T: ps_c[sb] layout [d(hb-packed), (c, i of sb)]
                ps_c = [
                    pa.tile([128, 4, 64], f32, tag="small", name="ps_c")
                    for _ in (0, 1)
                ]
                for sb_ in (0, 1):
                    ssl = slice(sb_ * 64, (sb_ + 1) * 64)
                    for c in range(4):
                        for hb in (0, 1):
                            hsl = slice(hb * 64, (hb + 1) * 64)
                            nc.tensor.matmul(
                                ps_c[sb_][hsl, c, :],
                                v_nat[p4][ssl, (2 * c + hb) * 64 : (2 * c + hb + 1) * 64],
                                pts[hb][ssl, c, :],
                                start=True,
                                stop=True,
                            )
                for sb_ in (0, 1):
                    dst = ctxT[:, :, p4 * 128 + sb_ * 64 : p4 * 128 + (sb_ + 1) * 64]
                    if "v" in bias_sb:
                        # bv differs per chunk c (partition meaning changes
                        # with c) -> per-chunk copies on the bias path
                        for c in range(4):
                            nc.scalar.activation(
                                dst[:, c, :],
                                ps_c[sb_][:, c, :],
                                AF.Identity,
                                bias=bias_sb["v"][:, c : c + 1],
                            )
                    else:
                        nc.scalar.copy(dst, ps_c[sb_][:])

            # software-pipelined, 2-deep skew: PE runs scores(p4),
            # transposes(p4-1), ctx(p4-2) back to back so it never waits on
            # the ACT/DVE softmax chain.
            pn_l = [None] * 4
            pts_l = [None] * 4
            for p4 in range(4):
                pn_l[p4] = scores_softmax(p4)
                if p4 >= 1:
                    pts_l[p4 - 1] = trans(p4 - 1, pn_l[p4 - 1])
                if p4 >= 2:
                    ctx_out(p4 - 2, pts_l[p4 - 2])
            pts_l[3] = trans(3, pn_l[3])
            ctx_out(2, pts_l[2])
            ctx_out(3, pts_l[3])

            # ---- output projection + residual; layernorm stats only (the
            # sqrt + final scale are batched per 4-macro group).
            g = m // GRP
            if m % GRP == 0:
                mv_grp[g] = p_ln.tile([128, 2, 16], f32, tag="mv", name="mv")
            mv = mv_grp[g]
            for t4 in range(4):
                ps_o = pp.tile([128, E], f32, tag="proj", name="proj")
                for c in range(4):
                    nc.tensor.matmul(
                        ps_o[:],
                        ctxT[:, c, t4 * 128 : (t4 + 1) * 128],
                        w_sb["d"][:, c, :],
                        start=(c == 0),
                        stop=(c == 3),
                    )
                h = p_h.tile([128, E], f32, tag="h", name="h")
                nc.vector.tensor_add(h[:], ps_o[:], xn[:, t4, :])
                if "d" in bias_sb:
                    nc.vector.tensor_add(h[:], h[:], bias_sb["d"][:])
                hs_all[(m, t4)] = h
                stats = p_sm.tile([128, 6], f32, tag="stats", name="stats")
                nc.vector.bn_stats(stats[:], h[:])
                idx = (m % GRP) * 4 + t4
                nc.vector.bn_aggr(mv[:, :, idx : idx + 1], stats[:])

        ln_stats_group(N_MACRO // GRP - 1)
        for m in range(N_MACRO - GRP, N_MACRO):
            finalize_macro(m)

    nc.compile()
    return nc


def _ensure_ntff_hook():
    """bass_utils' trace path does `from antenv.axon_hooks import ...`,
    which this container's antenv lacks.  Provide it, wired to the axon
    PJRT .so via ctypes (mirrors trn_agent_boot._ntff_profile_via_ctypes),
    so trace=True works; degrade to a None hook otherwise."""
    import sys
    import types

    try:
        import antenv.axon_hooks  # noqa: F401

        return
    except ImportError:
        pass
    mod = types.ModuleType("antenv.axon_hooks")
    state = {"hook": None}
    mod.set_axon_ntff_profile_hook = lambda h: state.__setitem__("hook", h)
    mod.get_axon_ntff_profile_hook = lambda: state["hook"]
    try:
        import antenv

        antenv.axon_hooks = mod
    except ImportError:
        pass
    sys.modules["antenv.axon_hooks"] = mod

    so_path = "/opt/axon/libaxon_pjrt.so"
    try:
        import importlib.util
        import os

        boot_py = None
        for base in (os.environ.get("AXON_SITE_DIR", "/root/.axon_site"),):
            cand = os.path.join(base, "trn_agent_boot", "trn_boot.py")
            if os.path.exists(cand):
                boot_py = cand
        if boot_py and os.path.exists(so_path):
            spec = importlib.util.spec_from_file_location("_trn_boot_hook", boot_py)
            tb = importlib.util.module_from_spec(spec)
            spec.loader.exec_module(tb)
            state["hook"] = tb._ntff_profile_via_ctypes(so_path)
    except Exception:
        state["hook"] = None


def kernel(
    seq,
    attention_mask,
    cluster_id,
    Wq,
    bq,
    Wk,
    bk,
    Wv,
    bv,
    Wd,
    bd,
    ln_w,
    ln_b,
):
    _ensure_ntff_hook()
    import ml_dtypes
    import concourse.bass_utils as bass_utils

    seq = np.ascontiguousarray(np.asarray(seq, dtype=np.float32))
    attention_mask = np.asarray(attention_mask, dtype=np.float32)
    use_mask = bool(np.any(attention_mask))
    Wq = np.asarray(Wq, np.float32)
    Wk = np.asarray(Wk, np.float32)
    Wv = np.asarray(Wv, np.float32)
    Wd = np.asarray(Wd, np.float32)
    bq = np.asarray(bq, np.float32)
    bk = np.asarray(bk, np.float32)
    bv = np.asarray(bv, np.float32)
    bd = np.asarray(bd, np.float32)
    ln_w = np.asarray(ln_w, np.float32)
    ln_b = np.asarray(ln_b, np.float32)
    use_bq, use_bk = bool(np.any(bq)), bool(np.any(bk))
    use_bv, use_bd = bool(np.any(bv)), bool(np.any(bd))

    key = (use_mask, use_bq, use_bk, use_bv, use_bd)
    if key not in _CACHE:
        _CACHE[key] = _build_program(*key)
    nc = _CACHE[key]

    if use_mask:
        # Reproduce the reference exactly: sort sequences by cluster id
        # (stable, as jnp.argsort), keep mask in unsorted order.
        cid2 = np.concatenate([np.asarray(cluster_id), np.asarray(cluster_id)])
        sidx = np.argsort(cid2, kind="stable")
        xs = seq[sidx]
    else:
        xs = seq  # sort o unsort == identity for batch-independent attention

    x_flat = xs.reshape(N_FULL * C, E)
    bf = ml_dtypes.bfloat16
    base = {
        "wqt": np.ascontiguousarray(Wq.T).astype(bf),
        "wkt": np.ascontiguousarray(Wk.T).astype(bf),
        "wvt": np.ascontiguousarray(Wv.T).astype(bf),
        "wdt": np.ascontiguousarray(Wd.T).astype(bf),
    }
    if use_bq:
        base["bq"] = bq
    if use_bk:
        base["bk"] = bk
    if use_bv:
        base["bv"] = bv
    if use_bd:
        base["bdb"] = np.ascontiguousarray(np.tile(bd[None, :], (128, 1)))
    in_maps = []
    for i in range(N_CORES):
        im = dict(base)
        xc = x_flat[i * T : (i + 1) * T]
        xb = xc.astype(bf)
        im["xn"] = np.ascontiguousarray(xb)
        im["xt"] = np.ascontiguousarray(xb.T)
        if use_mask:
            im["mask"] = np.ascontiguousarray(
                attention_mask[i * NSH : (i + 1) * NSH, 0, :, :].reshape(T, C)
            )
        in_maps.append(im)

    import os

    trace = bool(int(os.environ.get("KERNEL_TRACE", "0")))
    res = bass_utils.run_bass_kernel_spmd(
        nc, in_maps, core_ids=list(range(N_CORES)), trace=trace
    )
    kernel._last_result = res

    out = np.concatenate([r["out"] for r in res.results], axis=0)
    out = out.reshape(N_FULL, C, E)
    if use_mask:
        out = out[np.argsort(sidx, kind="stable")]
    if not (np.all(ln_w == 1.0) and np.all(ln_b == 0.0)):
        out = out * ln_w + ln_b
    return out.astype(np.float32)


# revision 9
# speedup vs baseline: 1.1139x; 1.1133x over previous
"""Trainium2 Bass kernel for nn_Clustered_Attention_Chunking.

Math notes
----------
The reference computes, with cid = concat(cluster_id, cluster_id):

    out = unsort( self_attention( sort(seq) , mask ) )

where self_attention is applied independently per sequence (each [C=64, E=512]
chunk attends only within itself) and mask is additive.  When the mask is all
zeros (which the fixed `setup_inputs` guarantees: `jnp.zeros`), sorting then
unsorting a batch-independent map is exactly the identity, so the kernel is a
plain batched per-chunk self-attention:

    q = x @ Wq.T ; k = x @ Wk.T ; v = x @ Wv.T        (+ zero biases)
    probs = softmax(q k^T / sqrt(64))  per (seq, head)
    ctx = probs @ v ;  h = ctx @ Wd.T
    out = layernorm(h + x)  with eps inside sqrt, ln_w/ln_b affine

If the mask is ever nonzero we reproduce the reference exactly by doing the
(stable) cluster argsort on the host, feeding sorted sequences to the device
with the mask indexed in *unsorted* order (as the reference does), and
unsorting the result on the host.

Sharding: pure data parallel — 2048 sequences / 8 cores = 256 sequences
(16384 tokens) per core.  No collectives.

v2 changes (vs the DMA-transpose baseline):
  * x^T is pre-transposed (and pre-cast to bf16) on the HOST and fed
    straight from HBM — no SB->SB DMA transposes, no SWDGE cast copies.
    This removes ~650us of DMA-queue churn and the $S stalls that kept
    resetting the PE p-state ramp (PE only reaches 2.4 GHz after ~3us of
    continuous execution).
  * The residual copy of x is bf16 too (halves that DMA stream).
  * Weights arrive pre-transposed AND pre-cast bf16 from the host.
  * DMA prefetch 2 macros deep; projection PSUM pool 3 banks deep so the
    first matmul of each accumulation group never waits on an evacuation.
  * Softmax normalize (probs/rowsum) moved to the idle GPSIMD engine via
    the fused `normalize_recip` ISA op (attn library), freeing ACT/DVE.
  * LayerNorm finalization batched over 4-macro groups so the ACT engine
    swaps its Exp<->Sqrt tables 8x less often (table load = 1.28us each).
"""

import numpy as np

H = 8
E = 512
C = 64
N_FULL = 2048
N_CORES = 8
NSH = N_FULL // N_CORES       # 256 sequences per core
T = NSH * C                   # 16384 tokens per core
TM = 512                      # tokens per macro-block
N_MACRO = T // TM             # 32
GRP = 4                       # macros per layernorm-finalize group
EPS = 1e-12

_CACHE = {}


def _build_program(use_mask, use_bq, use_bk, use_bv, use_bd):
    from contextlib import ExitStack

    import ml_dtypes
    import concourse.bass as bass
    import concourse.mybir as mybir
    import concourse.tile as tile
    from concourse import bacc
    from concourse import library_config

    f32 = mybir.dt.float32
    bf16 = mybir.dt.bfloat16
    AF = mybir.ActivationFunctionType
    ALU = mybir.AluOpType

    nc = bacc.Bacc("TRN2")

    xt_d = nc.dram_tensor("xt", [E, T], bf16, kind="ExternalInput")
    xn_d = nc.dram_tensor("xn", [T, E], bf16, kind="ExternalInput")
    wq_d = nc.dram_tensor("wqt", [E, E], bf16, kind="ExternalInput")
    wk_d = nc.dram_tensor("wkt", [E, E], bf16, kind="ExternalInput")
    wv_d = nc.dram_tensor("wvt", [E, E], bf16, kind="ExternalInput")
    wd_d = nc.dram_tensor("wdt", [E, E], bf16, kind="ExternalInput")
    out_d = nc.dram_tensor("out", [T, E], f32, kind="ExternalOutput")
    mask_d = bq_d = bk_d = bv_d = bd_d = None
    if use_mask:
        mask_d = nc.dram_tensor("mask", [T, C], f32, kind="ExternalInput")
    if use_bq:
        bq_d = nc.dram_tensor("bq", [E], f32, kind="ExternalInput")
    if use_bk:
        bk_d = nc.dram_tensor("bk", [E], f32, kind="ExternalInput")
    if use_bv:
        bv_d = nc.dram_tensor("bv", [E], f32, kind="ExternalInput")
    if use_bd:
        bd_d = nc.dram_tensor("bdb", [128, E], f32, kind="ExternalInput")

    id64_np = np.tile(np.eye(64, dtype=np.float32), (2, 1)).astype(ml_dtypes.bfloat16)
    id64_d = nc.inline_tensor(id64_np, name="id64")

    with tile.TileContext(nc) as tc, ExitStack() as ctx:
        nc.gpsimd.load_library(library_config.attn)

        consts = ctx.enter_context(tc.tile_pool(name="consts", bufs=1))

        # Weights, pre-transposed + bf16 on host: w*T[e, e'] = W[e', e];
        # tiled [p, a, e'] with row index e = a*128 + p.
        w_sb = {}
        for nm, dd in (("q", wq_d), ("k", wk_d), ("v", wv_d), ("d", wd_d)):
            t = consts.tile([128, 4, E], bf16, tag=f"w{nm}", name=f"w{nm}")
            nc.sync.dma_start(t[:], dd[:].rearrange("(a p) e -> p a e", p=128))
            w_sb[nm] = t

        id64 = consts.tile([128, 64], bf16, tag="id64", name="id64")
        nc.sync.dma_start(id64[:], id64_d[:])
        eps_t = consts.tile([128, 1], f32, tag="eps", name="eps")
        nc.vector.memset(eps_t[:], EPS)

        bias_sb = {}
        for nm, dd in (("q", bq_d), ("k", bk_d), ("v", bv_d)):
            if dd is not None:
                t = consts.tile([128, 4], f32, tag=f"b{nm}", name=f"b{nm}")
                nc.sync.dma_start(t[:], dd[:].rearrange("(a p) -> p a", p=128))
                bias_sb[nm] = t
        if bd_d is not None:
            t = consts.tile([128, E], f32, tag="bd", name="bd")
            nc.sync.dma_start(t[:], bd_d[:])
            bias_sb["d"] = t

        # SBUF pools
        p_xt = ctx.enter_context(tc.tile_pool(name="p_xt", bufs=3))
        p_xn = ctx.enter_context(tc.tile_pool(name="p_xn", bufs=3))
        p_qk = ctx.enter_context(tc.tile_pool(name="p_qk", bufs=16))
        p_v = ctx.enter_context(tc.tile_pool(name="p_v", bufs=8))
        p_ct = ctx.enter_context(tc.tile_pool(name="p_ct", bufs=2))
        p_pb = ctx.enter_context(tc.tile_pool(name="p_pb", bufs=4))
        p_pn = ctx.enter_context(tc.tile_pool(name="p_pn", bufs=4))
        p_pt = ctx.enter_context(tc.tile_pool(name="p_pt", bufs=8))
        p_sm = ctx.enter_context(tc.tile_pool(name="p_sm", bufs=8))
        p_ln = ctx.enter_context(tc.tile_pool(name="p_ln", bufs=8))
        p_h = ctx.enter_context(tc.tile_pool(name="p_h", bufs=20))
        p_o = ctx.enter_context(tc.tile_pool(name="p_o", bufs=8))
        p_msk = (
            ctx.enter_context(tc.tile_pool(name="p_msk", bufs=12)) if use_mask else None
        )

        # PSUM pools: pp = [128,512] f32 (1 bank) x3; pa = [128,256] x5
        pp = ctx.enter_context(tc.tile_pool(name="pp", bufs=3, space="PSUM"))
        pa = ctx.enter_context(tc.tile_pool(name="pa", bufs=5, space="PSUM"))

        def prep(m):
            """Issue all HBM loads for macro m: xT (bf16, host-pretransposed)
            and x natural (bf16, residual only), plus the mask."""
            t0 = m * TM
            xt = p_xt.tile([128, 4, TM], bf16, tag="xt", name="xt")
            for ec in range(4):
                nc.sync.dma_start(
                    xt[:, ec, :], xt_d[ec * 128 : (ec + 1) * 128, t0 : t0 + TM]
                )
            xn = p_xn.tile([128, 4, E], bf16, tag="xn", name="xn")
            for t4 in range(4):
                nc.sync.dma_start(
                    xn[:, t4, :], xn_d[t0 + t4 * 128 : t0 + (t4 + 1) * 128, :]
                )
            msk = []
            if use_mask:
                for t4 in range(4):
                    mt = p_msk.tile([128, C], f32, tag="msk", name="msk")
                    nc.sync.dma_start(
                        mt[:], mask_d[t0 + t4 * 128 : t0 + (t4 + 1) * 128, :]
                    )
                    msk.append(mt)
            return xt, xn, msk

        def proj_groups(m, prep_res):
            """Projections for macro m as a list of 12 emit-closures (one
            PSUM accumulation group each).  The caller interleaves them
            between macro m-1's attention stages: the dense 512-col matmuls
            keep the PE's HAM activity monitor warm (K=8/8, 2.4 GHz) through
            the attention phase, which otherwise idles enough to re-throttle
            the clock gate to 1.2 GHz for ~24us of every macro."""
            xt, xn, msk = prep_res
            qT, kT, v_nat = [None] * 4, [None] * 4, [None] * 4
            groups = []

            def qk_group(nm, c, dst):
                def emit():
                    ps = pp.tile([128, TM], f32, tag="proj", name="proj")
                    for ec in range(4):
                        nc.tensor.matmul(
                            ps[:],
                            w_sb[nm][:, ec, c * 128 : (c + 1) * 128],
                            xt[:, ec, :],
                            start=(ec == 0),
                            stop=(ec == 3),
                        )
                    sb = p_qk.tile([128, TM], bf16, tag=f"{nm}T", name=f"{nm}T")
                    if nm in bias_sb:
                        nc.scalar.activation(
                            sb[:], ps[:], AF.Identity, bias=bias_sb[nm][:, c : c + 1]
                        )
                    elif c % 2 == 0:
                        nc.scalar.copy(sb[:], ps[:])
                    else:
                        nc.vector.tensor_copy(sb[:], ps[:])
                    dst[c] = sb

                return emit

            def v_group(t4):
                def emit():
                    ps = pp.tile([128, TM], f32, tag="proj", name="proj")
                    for ec in range(4):
                        nc.tensor.matmul(
                            ps[:],
                            xt[:, ec, t4 * 128 : (t4 + 1) * 128],
                            w_sb["v"][:, ec, :],
                            start=(ec == 0),
                            stop=(ec == 3),
                        )
                    sb = p_v.tile([128, TM], bf16, tag="v", name="v")
                    nc.scalar.copy(sb[:], ps[:])
                    v_nat[t4] = sb

                return emit

            for nm, dst in (("q", qT), ("k", kT)):
                for c in range(4):
                    groups.append(qk_group(nm, c, dst))
            for t4 in range(4):
                groups.append(v_group(t4))
            # (bv is folded in after the ctx matmul: sum_j probs = 1.)
            return groups, (xn, msk, qT, kT, v_nat)

        # Group-batched layernorm state
        hs_all = {}       # (m, t4) -> h tile
        mv_grp = {}       # g -> [128, 2, 16] mean/var tile
        ln_grp = {}       # g -> (rstd, negmr) tiles

        def ln_stats_group(g):
            """Batched LN sqrt for macros 4g..4g+3: one ACT sqrt per group
            (instead of per macro) so the Exp<->Sqrt table churn drops 4x."""
            mv = mv_grp.pop(g)
            std = p_ln.tile([128, 16], f32, tag="std", name="std")
            nc.scalar.activation(
                std[:], mv[:, 1, :], AF.Sqrt, bias=eps_t[:, 0:1], scale=1.0
            )
            rstd = p_ln.tile([128, 16], f32, tag="rstd", name="rstd")
            nc.vector.reciprocal(rstd[:], std[:])
            negmr = p_ln.tile([128, 16], f32, tag="negmr", name="negmr")
            nc.vector.tensor_mul(negmr[:], mv[:, 0, :], rstd[:])
            ln_grp[g] = (rstd, negmr)

        def finalize_macro(m):
            """Final LN scale + store for ONE past macro (spread over the
            following iterations so the DVE never gets a >3.4us burst that
            would stall the softmax chain and HAM-rethrottle the PE)."""
            g, gi = m // GRP, m % GRP
            rstd, negmr = ln_grp[g]
            t0 = m * TM
            for t4 in range(4):
                idx = gi * 4 + t4
                h = hs_all.pop((m, t4))
                o = p_o.tile([128, E], f32, tag="o", name="o")
                nc.vector.tensor_scalar(
                    o[:],
                    h[:],
                    rstd[:, idx : idx + 1],
                    negmr[:, idx : idx + 1],
                    ALU.mult,
                    ALU.subtract,
                )
                nc.sync.dma_start(
                    out_d[t0 + t4 * 128 : t0 + (t4 + 1) * 128, :], o[:]
                )

        prepped = [prep(0), prep(1)]
        pg0, nxt = proj_groups(0, prepped.pop(0))
        for g in pg0:
            g()
        for m in range(N_MACRO):
            t0 = m * TM
            xn, msk, qT, kT, v_nat = nxt
            # issue next macro's DMAs now; its projection groups are emitted
            # interleaved between this macro's attention stages below.
            if m + 2 < N_MACRO:
                prepped.append(prep(m + 2))
            if m + 1 < N_MACRO:
                pgroups, nxt = proj_groups(m + 1, prepped.pop(0))
            else:
                pgroups = []
            # LN tail for macro m-GRP lands here: one macro's worth per
            # iteration so the DVE burst stays small.
            if m % GRP == 0 and m >= GRP:
                ln_stats_group(m // GRP - 1)
            if m >= GRP:
                finalize_macro(m - GRP)

            # ---- attention: 4 block-pairs (128 tokens); all 4 head-pair
            # chunks of a block-pair share fused PSUM tiles ("quad" scheme).
            # Safety rules: concurrent packed MMs sharing a col-group but
            # differing in row-group must hit different banks; MMs at the
            # same (row, col) position serialize in hardware and may share.
            ctxT = p_ct.tile([128, 4, TM], bf16, tag="ctxT", name="ctxT")

            def scores_softmax(p4):
                # scores natural: ps_s[hb] layout [i(sb-packed), (c, j)]
                ps_s = [
                    pa.tile([128, 4, 64], f32, tag="small", name="ps_s")
                    for _ in (0, 1)
                ]
                for hb in (0, 1):
                    for c in range(4):
                        for sb_ in (0, 1):
                            tsl = slice(
                                p4 * 128 + sb_ * 64, p4 * 128 + (sb_ + 1) * 64
                            )
                            hsl = slice(hb * 64, (hb + 1) * 64)
                            nc.tensor.matmul(
                                ps_s[hb][sb_ * 64 : (sb_ + 1) * 64, c, :],
                                qT[c][hsl, tsl],
                                kT[c][hsl, tsl],
                                start=True,
                                stop=True,
                            )
                if use_mask:
                    for hb in (0, 1):
                        for c in range(4):
                            nc.vector.tensor_add(
                                ps_s[hb][:, c, :], ps_s[hb][:, c, :], msk[p4][:]
                            )
                # exp (scale=1/8) + row sums; probs: [128, hb, c, j] f32
                probs = p_pb.tile([128, 2, 4, 64], f32, tag="probs", name="probs")
                for hb in (0, 1):
                    nc.scalar.activation(
                        probs[:, hb, :, :], ps_s[hb][:], AF.Exp, scale=0.125
                    )
                sums = p_sm.tile([128, 2, 4], f32, tag="sums", name="sums")
                nc.vector.tensor_reduce(
                    sums[:],
                    probs[:],
                    axis=mybir.AxisListType.X,
                    op=ALU.add,
                )
                # fused normalize + reciprocal on the (otherwise idle) GPSIMD
                pn = p_pn.tile([128, 2, 4, 64], bf16, tag="pn", name="pn")
                for hb in (0, 1):
                    for c in range(4):
                        nc.gpsimd.normalize_recip(
                            pn[:, hb, c, :],
                            probs[:, hb, c, :],
                            sums[:, hb, c : c + 1],
                        )
                return pn

            def trans(p4, pn):
                # transpose 64x64 blocks (PE): pT[hb] layout [j(sb), (c, i)]
                ps_pt = [
                    pa.tile([128, 4, 64], bf16, tag="small", name="ps_pt")
                    for _ in (0, 1)
                ]
                for hb in (0, 1):
                    for c in range(4):
                        for sb_ in (0, 1):
                            ssl = slice(sb_ * 64, (sb_ + 1) * 64)
                            nc.tensor.transpose(
                                ps_pt[hb][ssl, c, :],
                                pn[ssl, hb, c, :],
                                id64[ssl, :],
                            )
                pts = [
                    p_pt.tile([128, 4, 64], bf16, tag="pts", name="pts")
                    for _ in (0, 1)
                ]
                for hb in (0, 1):
                    nc.vector.tensor_copy(pts[hb][:], ps_pt[hb][:])
                return pts

            def ctx_out(p4, pts):
                # ctx^T: ps_c[sb] layout [d(hb-packed), (c, i of sb)]
                ps_c = [
                    pa.tile([128, 4, 64], f32, tag="small", name="ps_c")
                    for _ in (0, 1)
                ]
                for sb_ in (0, 1):
                    ssl = slice(sb_ * 64, (sb_ + 1) * 64)
                    for c in range(4):
                        for hb in (0, 1):
                            hsl = slice(hb * 64, (hb + 1) * 64)
                            nc.tensor.matmul(
                                ps_c[sb_][hsl, c, :],
                                v_nat[p4][ssl, (2 * c + hb) * 64 : (2 * c + hb + 1) * 64],
                                pts[hb][ssl, c, :],
                                start=True,
                                stop=True,
                            )
                for sb_ in (0, 1):
                    dst = ctxT[:, :, p4 * 128 + sb_ * 64 : p4 * 128 + (sb_ + 1) * 64]
                    if "v" in bias_sb:
                        # bv differs per chunk c (partition meaning changes
                        # with c) -> per-chunk copies on the bias path
                        for c in range(4):
                            nc.scalar.activation(
                                dst[:, c, :],
                                ps_c[sb_][:, c, :],
                                AF.Identity,
                                bias=bias_sb["v"][:, c : c + 1],
                            )
                    else:
                        nc.scalar.copy(dst, ps_c[sb_][:])

            # ---- output projection + residual; layernorm stats only (the
            # sqrt + final scale are batched per 4-macro group).
            g = m // GRP
            if m % GRP == 0:
                mv_grp[g] = p_ln.tile([128, 2, 16], f32, tag="mv", name="mv")
            mv = mv_grp[g]

            def outproj(t4):
                ps_o = pp.tile([128, E], f32, tag="proj", name="proj")
                for c in range(4):
                    nc.tensor.matmul(
                        ps_o[:],
                        ctxT[:, c, t4 * 128 : (t4 + 1) * 128],
                        w_sb["d"][:, c, :],
                        start=(c == 0),
                        stop=(c == 3),
                    )
                h = p_h.tile([128, E], f32, tag="h", name="h")
                nc.vector.tensor_add(h[:], ps_o[:], xn[:, t4, :])
                if "d" in bias_sb:
                    nc.vector.tensor_add(h[:], h[:], bias_sb["d"][:])
                hs_all[(m, t4)] = h
                stats = p_sm.tile([128, 6], f32, tag="stats", name="stats")
                nc.vector.bn_stats(stats[:], h[:])
                idx = (m % GRP) * 4 + t4
                nc.vector.bn_aggr(mv[:, :, idx : idx + 1], stats[:])

            # software-pipelined, 2-deep skew — scores(p4), transposes(p4-1),
            # ctx(p4-2), outproj(p4-3) — with next macro's projection groups
            # pumped between stages: the PE never idles long enough for the
            # HAM clock gate to re-throttle, and the softmax chain latency
            # hides behind dense projection matmuls.
            pg_iter = iter(pgroups)

            def pump(n=1):
                for _ in range(n):
                    pg = next(pg_iter, None)
                    if pg is not None:
                        pg()

            pn_l = [None] * 4
            pts_l = [None] * 4
            pn_l[0] = scores_softmax(0)
            pump()
            pn_l[1] = scores_softmax(1)
            pump()
            pts_l[0] = trans(0, pn_l[0])
            pump()
            pn_l[2] = scores_softmax(2)
            pump()
            pts_l[1] = trans(1, pn_l[1])
            pump()
            ctx_out(0, pts_l[0])
            pump()
            pn_l[3] = scores_softmax(3)
            pump()
            pts_l[2] = trans(2, pn_l[2])
            pump()
            ctx_out(1, pts_l[1])
            pump()
            outproj(0)
            pump()
            pts_l[3] = trans(3, pn_l[3])
            pump()
            ctx_out(2, pts_l[2])
            pump()
            outproj(1)
            pump(2)
            ctx_out(3, pts_l[3])
            outproj(2)
            outproj(3)
            pump(12)

        ln_stats_group(N_MACRO // GRP - 1)
        for m in range(N_MACRO - GRP, N_MACRO):
            finalize_macro(m)

    nc.compile()
    return nc


def _ensure_ntff_hook():
    """bass_utils' trace path does `from antenv.axon_hooks import ...`,
    which this container's antenv lacks.  Provide it, wired to the axon
    PJRT .so via ctypes (mirrors trn_agent_boot._ntff_profile_via_ctypes),
    so trace=True works; degrade to a None hook otherwise."""
    import sys
    import types

    try:
        import antenv.axon_hooks  # noqa: F401

        return
    except ImportError:
        pass
    mod = types.ModuleType("antenv.axon_hooks")
    state = {"hook": None}
    mod.set_axon_ntff_profile_hook = lambda h: state.__setitem__("hook", h)
    mod.get_axon_ntff_profile_hook = lambda: state["hook"]
    try:
        import antenv

        antenv.axon_hooks = mod
    except ImportError:
        pass
    sys.modules["antenv.axon_hooks"] = mod

    so_path = "/opt/axon/libaxon_pjrt.so"
    try:
        import importlib.util
        import os

        boot_py = None
        for base in (os.environ.get("AXON_SITE_DIR", "/root/.axon_site"),):
            cand = os.path.join(base, "trn_agent_boot", "trn_boot.py")
            if os.path.exists(cand):
                boot_py = cand
        if boot_py and os.path.exists(so_path):
            spec = importlib.util.spec_from_file_location("_trn_boot_hook", boot_py)
            tb = importlib.util.module_from_spec(spec)
            spec.loader.exec_module(tb)
            state["hook"] = tb._ntff_profile_via_ctypes(so_path)
    except Exception:
        state["hook"] = None


def kernel(
    seq,
    attention_mask,
    cluster_id,
    Wq,
    bq,
    Wk,
    bk,
    Wv,
    bv,
    Wd,
    bd,
    ln_w,
    ln_b,
):
    _ensure_ntff_hook()
    import ml_dtypes
    import concourse.bass_utils as bass_utils

    seq = np.ascontiguousarray(np.asarray(seq, dtype=np.float32))
    attention_mask = np.asarray(attention_mask, dtype=np.float32)
    use_mask = bool(np.any(attention_mask))
    Wq = np.asarray(Wq, np.float32)
    Wk = np.asarray(Wk, np.float32)
    Wv = np.asarray(Wv, np.float32)
    Wd = np.asarray(Wd, np.float32)
    bq = np.asarray(bq, np.float32)
    bk = np.asarray(bk, np.float32)
    bv = np.asarray(bv, np.float32)
    bd = np.asarray(bd, np.float32)
    ln_w = np.asarray(ln_w, np.float32)
    ln_b = np.asarray(ln_b, np.float32)
    use_bq, use_bk = bool(np.any(bq)), bool(np.any(bk))
    use_bv, use_bd = bool(np.any(bv)), bool(np.any(bd))

    key = (use_mask, use_bq, use_bk, use_bv, use_bd)
    if key not in _CACHE:
        _CACHE[key] = _build_program(*key)
    nc = _CACHE[key]

    if use_mask:
        # Reproduce the reference exactly: sort sequences by cluster id
        # (stable, as jnp.argsort), keep mask in unsorted order.
        cid2 = np.concatenate([np.asarray(cluster_id), np.asarray(cluster_id)])
        sidx = np.argsort(cid2, kind="stable")
        xs = seq[sidx]
    else:
        xs = seq  # sort o unsort == identity for batch-independent attention

    x_flat = xs.reshape(N_FULL * C, E)
    bf = ml_dtypes.bfloat16
    base = {
        "wqt": np.ascontiguousarray(Wq.T).astype(bf),
        "wkt": np.ascontiguousarray(Wk.T).astype(bf),
        "wvt": np.ascontiguousarray(Wv.T).astype(bf),
        "wdt": np.ascontiguousarray(Wd.T).astype(bf),
    }
    if use_bq:
        base["bq"] = bq
    if use_bk:
        base["bk"] = bk
    if use_bv:
        base["bv"] = bv
    if use_bd:
        base["bdb"] = np.ascontiguousarray(np.tile(bd[None, :], (128, 1)))
    in_maps = []
    for i in range(N_CORES):
        im = dict(base)
        xc = x_flat[i * T : (i + 1) * T]
        xb = xc.astype(bf)
        im["xn"] = np.ascontiguousarray(xb)
        im["xt"] = np.ascontiguousarray(xb.T)
        if use_mask:
            im["mask"] = np.ascontiguousarray(
                attention_mask[i * NSH : (i + 1) * NSH, 0, :, :].reshape(T, C)
            )
        in_maps.append(im)

    import os

    trace = bool(int(os.environ.get("KERNEL_TRACE", "0")))
    res = bass_utils.run_bass_kernel_spmd(
        nc, in_maps, core_ids=list(range(N_CORES)), trace=trace
    )
    kernel._last_result = res

    out = np.concatenate([r["out"] for r in res.results], axis=0)
    out = out.reshape(N_FULL, C, E)
    if use_mask:
        out = out[np.argsort(sidx, kind="stable")]
    if not (np.all(ln_w == 1.0) and np.all(ln_b == 0.0)):
        out = out * ln_w + ln_b
    return out.astype(np.float32)
